# revision 1
# baseline (speedup 1.0000x reference)
"""Trainium2 Bass kernel for nn_InteractionGate (gnn_message_passing).

Contract: kernel(**inputs) takes the FULL unsharded inputs (as in
reference.setup_inputs()) and returns the FULL [1024, 1024, 64] output.
Internally shards the pairwise row dimension i across 8 NeuronCores
(128 rows each), runs one SPMD Bass/Tile program on cores 0-7, gathers.

Math (see derivation in comments): with
  W1 = w_gate[0:64], W2 = w_gate[64:128], W3 = w_gate[128:144], W4 = w_gate[144:160]
  u3 = w_dist @ W3, u4 = w_dist @ W4
  B  = AH @ (W1+W2) + b_dist @ (W3+W4) + b_gate          [N,H]
the reference reduces (off-diagonal) to
  out[i,j,h] = AH[j,h] * sigmoid(B[j,h] + diagv[i]*u3[h] + dist[i,j]*u4[h])
where dist is the cal_dist "distance_other" matrix. The diagonal entries
use GH instead of AH and are patched on the host (O(N*H) work).

Device plan per core (rows i in its 128-block, partitions = i):
  1. PE computes the five pairwise numerator matrices (each is rank<=6:
     sum_k f_k(i) g_k(j)) as K=6 matmuls.
  2. DVE/ACT compute dist[i,j] [128,1024] elementwise (reciprocal, sqrt,
     branch masks via predicated copies).
  3. PE transposes dist into dT33 [33, 4096] (row 32 = diagv row) via 8
     128x128 transposes + 4 SBUF->SBUF relayout DMAs.
  4. Main loop over 64 half-chunks (16 j's x 64 h = 1024 free each):
     PE:  logit = dT33-chunk.T @ [delta*u4 ; u3row]  (K=33)
                + onehot.T @ B_all   (K=32 one-hot row-selector broadcast)
          ah    = onehot.T @ AH_all  (K=32)
     ACT: sig = sigmoid(logit)   (PSUM -> SBUF)
     DVE: out = sig * ah         (SBUF x PSUM -> SBUF)
     DMA: out tile (4 half-chunks batched = 2 MiB) -> HBM.
"""
import os
import sys
from contextlib import ExitStack

import numpy as np

if "/opt/trn_rl_repo" not in sys.path:
    sys.path.insert(0, "/opt/trn_rl_repo")

import concourse.bass as bass
import concourse.bacc as bacc
import concourse.mybir as mybir
import concourse.tile as tile
from concourse import bass_utils

N, H, E = 1024, 64, 16
NCORES = 8
R = N // NCORES            # 128 rows per core
F32 = mybir.dt.float32
AF = mybir.ActivationFunctionType
OP = mybir.AluOpType

NJ_CHUNK = 32              # j's per K-matmul chunk (lhsT partition rows)
NCHUNK = N // NJ_CHUNK     # 32 chunks
NJ_HALF = 16               # j's per PSUM window
WFREE = NJ_HALF * H        # 1024 free elements per window
NHALF = N // NJ_HALF       # 64 windows per core
OUT_BATCH = 2              # windows per output DMA (1 MiB per DMA)


def _sigmoid(x):
    return 1.0 / (1.0 + np.exp(-x))


def _host_prep(action_hidden_state, goal_hidden_state, goal, action,
               w_dist, b_dist, w_gate, b_gate):
    f32 = np.float32
    AH = np.ascontiguousarray(action_hidden_state, f32)
    GH = np.ascontiguousarray(goal_hidden_state, f32)
    goal = np.asarray(goal, f32)
    action = np.asarray(action, f32)
    w_dist = np.asarray(w_dist, f32)
    b_dist = np.asarray(b_dist, f32)
    w_gate = np.asarray(w_gate, f32)
    b_gate = np.asarray(b_gate, f32)

    ax, ay = action[:, 0].copy(), action[:, 1].copy()
    gx, gy = goal[:, 0].copy(), goal[:, 1].copy()
    gyx = gy - gx
    diagv = np.sqrt((ax - gx) ** 2 + (ay - gy) ** 2).astype(f32)

    W1, W2 = w_gate[0:H], w_gate[H:2 * H]
    W3, W4 = w_gate[2 * H:2 * H + E], w_gate[2 * H + E:2 * H + 2 * E]
    u3 = (w_dist @ W3).astype(f32)
    u4 = (w_dist @ W4).astype(f32)
    B = (AH @ (W1 + W2) + b_dist @ (W3 + W4) + b_gate).astype(f32)

    one = np.ones(N, f32)
    # rank factors: num[i,j] = sum_k f[k][i] * g[k][j]
    f_cav = np.stack([ax, -ax * gx, -ay, ay * gx])
    g_cav = np.stack([ay * gx, ay, ax * gx, ax])
    f_caz = np.stack([ax, -ax * gy, -ay, ay * gy])
    g_caz = np.stack([ay * gy, ay, ax * gy, ax])
    f_wcg1 = np.stack([gx, -ax * gx]); g_wcg1 = np.stack([ax * gyx, gyx])
    f_wcg2 = np.stack([gyx, -ax * gyx]); g_wcg2 = np.stack([ax * gx, gx])
    f_scg1 = np.stack([gx, -ay * gx]); g_scg1 = np.stack([ax * gyx, gyx])
    f_t2 = np.stack([gyx, -ax * gyx]); g_t2 = np.stack([ay * gx, gx])
    f_dnm = np.stack([one, -ay, -gx, ay * gx, np.zeros(N, f32), np.zeros(N, f32)])
    g_dnm = np.stack([ay * gx, gx, ay, one, np.zeros(N, f32), np.zeros(N, f32)])

    fg = dict(
        dnm=(f_dnm, g_dnm),
        num1=(np.concatenate([f_cav, -f_wcg1]), np.concatenate([g_cav, g_wcg1])),
        num1p=(np.concatenate([f_cav, f_wcg2]), np.concatenate([g_cav, g_wcg2])),
        num2=(np.concatenate([f_caz, -f_scg1]), np.concatenate([g_caz, g_scg1])),
        num2p=(np.concatenate([f_caz, f_t2]), np.concatenate([g_caz, g_t2])),
    )

    logit_diag = (B + (GH - AH) @ W2 + diagv[:, None] * (u3 + u4)).astype(f32)
    out_diag = (GH * _sigmoid(logit_diag)).astype(f32)

    return dict(AH=AH, GH=GH, ax=ax, ay=ay, gx=gx, gy=gy, diagv=diagv,
                u3=u3, u4=u4, B=B, fg=fg, out_diag=out_diag)


NUM_NAMES = ["dnm", "num1", "num1p", "num2", "num2p"]


def _core_inputs(prep, core):
    """Build the per-core in_map (numpy arrays for every ExternalInput)."""
    f32 = np.float32
    i0 = core * R
    sl = slice(i0, i0 + R)

    sc = np.zeros((R, 8), f32)
    sc[:, 0] = prep["ax"][sl]
    sc[:, 1] = prep["ay"][sl]
    sc[:, 2] = prep["gx"][sl]
    sc[:, 3] = prep["diagv"][sl]

    jj = np.arange(N)[None, :]
    ii = np.arange(i0, i0 + R)[:, None]
    mju = (jj > ii).astype(f32)
    mjl = (jj < ii).astype(f32)

    axj_b = np.broadcast_to(prep["ax"], (R, N)).copy()
    gxj_b = np.broadcast_to(prep["gx"], (R, N)).copy()

    ident = np.eye(128, dtype=f32)

    dvi_rep = np.tile(prep["diagv"][sl], NCHUNK)[None, :]  # [1, 32*128]

    # rhs33: rows 0..31 delta(j_local)*u4 over a 32-j chunk, row 32 = u3 tiled
    rhs33 = np.zeros((33, NJ_CHUNK * H), f32)
    for jl in range(NJ_CHUNK):
        rhs33[jl, jl * H:(jl + 1) * H] = prep["u4"]
    rhs33[32] = np.tile(prep["u3"], NJ_CHUNK)

    # one-hot row selector blocks: onehot[k, r*128+m] = (k==r)
    onehot = np.zeros((32, 32 * 128), f32)
    for r_ in range(32):
        onehot[r_, r_ * 128:(r_ + 1) * 128] = 1.0

    # B_all/AH_all: window hh data at [hh%32, (hh//32)*WFREE : +WFREE]
    B_all = np.zeros((32, 2 * WFREE), f32)
    AH_all = np.zeros((32, 2 * WFREE), f32)
    for hh in range(NHALF):
        r_, q_ = hh % 32, hh // 32
        seg = slice(q_ * WFREE, (q_ + 1) * WFREE)
        B_all[r_, seg] = prep["B"][hh * NJ_HALF:(hh + 1) * NJ_HALF].reshape(-1)
        AH_all[r_, seg] = prep["AH"][hh * NJ_HALF:(hh + 1) * NJ_HALF].reshape(-1)

    meye = (jj == ii).astype(f32)
    m = dict(sc=sc, mju=mju, mjl=mjl, meye=meye, axj_b=axj_b, gxj_b=gxj_b, ident=ident,
             dvi_rep=dvi_rep, rhs33=rhs33, onehot=onehot,
             B_all=B_all, AH_all=AH_all)
    for nm in NUM_NAMES:
        f, g = prep["fg"][nm]
        m[f"lhsT_{nm}"] = np.ascontiguousarray(f[:, sl].astype(f32))  # [6, 128]
        m[f"rhs_{nm}"] = np.ascontiguousarray(g.astype(f32))          # [6, 1024]
    return m


def _declare_tensors(nc):
    t = {}
    def inp(name, shape):
        t[name] = nc.dram_tensor(name, shape, F32, kind="ExternalInput").ap()
    inp("sc", [R, 8])
    inp("mju", [R, N]); inp("mjl", [R, N]); inp("meye", [R, N])
    inp("axj_b", [R, N]); inp("gxj_b", [R, N])
    inp("ident", [128, 128])
    inp("dvi_rep", [1, NCHUNK * 128])
    inp("rhs33", [33, NJ_CHUNK * H])
    inp("onehot", [32, 32 * 128])
    inp("B_all", [32, 2 * WFREE]); inp("AH_all", [32, 2 * WFREE])
    for nm in NUM_NAMES:
        inp(f"lhsT_{nm}", [6, 128])
        inp(f"rhs_{nm}", [6, N])
    t["out"] = nc.dram_tensor("out", [R, N * H], F32, kind="ExternalOutput").ap()
    return t


def _build_program(ctx, tc, t):
    nc = tc.nc

    consts = ctx.enter_context(tc.tile_pool(name="consts", bufs=1))

    def load(name, shape):
        tl = consts.tile(shape, F32, tag=name, name=name)
        nc.sync.dma_start(tl[:], t[name])
        return tl

    sc = load("sc", [R, 8])
    mju = load("mju", [R, N])
    mjl = load("mjl", [R, N])
    meye = load("meye", [R, N])
    axj_b = load("axj_b", [R, N])
    gxj_b = load("gxj_b", [R, N])
    ident = load("ident", [128, 128])
    rhs33 = load("rhs33", [33, NJ_CHUNK * H])
    onehot = load("onehot", [32, 32 * 128])
    B_all = load("B_all", [32, 2 * WFREE])
    AH_all = load("AH_all", [32, 2 * WFREE])
    lhsT_num = {nm: load(f"lhsT_{nm}", [6, 128]) for nm in NUM_NAMES}
    rhs_num = {nm: load(f"rhs_{nm}", [6, N]) for nm in NUM_NAMES}

    AXi, AYi, GXi, DVi = (sc[:, k:k + 1] for k in range(4))

    # ---- phase 1: numerators via PE (rank<=6), eviction to SBUF ----
    nums = ctx.enter_context(tc.tile_pool(name="nums", bufs=1))
    work = ctx.enter_context(tc.tile_pool(name="work", bufs=1))
    num_sb = {}
    with tc.tile_pool(name="ps_num", bufs=2, space="PSUM") as ps_num:
        for nm in NUM_NAMES:
            ps = ps_num.tile([R, N], F32, tag="ps_num", name="ps_num")
            for w in range(N // 512):
                nc.tensor.matmul(ps[:, w * 512:(w + 1) * 512],
                                 lhsT_num[nm][:, :],
                                 rhs_num[nm][:, w * 512:(w + 1) * 512],
                                 start=True, stop=True)
            sb = nums.tile([R, N], F32, tag=f"num_{nm}", name=f"num_{nm}")
            nc.vector.tensor_copy(sb[:], ps[:])
            num_sb[nm] = sb

    # ---- phase 2: dist [128, 1024] elementwise ----
    # scratch slots rotate through one tag; peak liveness ~6
    def wtile():
        return work.tile([R, N], F32, tag="w", name="w", bufs=8)

    # p's overwrite their numerator tiles in place; rdn overwrites dnm
    rdn = num_sb["dnm"]
    nc.gpsimd.tensor_add(rdn[:], rdn[:], meye[:])
    nc.vector.reciprocal(rdn[:], rdn[:])
    p1, p2, p1p, p2p = (num_sb[k] for k in ("num1", "num2", "num1p", "num2p"))
    nc.vector.tensor_mul(p1[:], p1[:], rdn[:])
    nc.vector.tensor_mul(p2[:], p2[:], rdn[:])
    nc.vector.tensor_mul(p1p[:], p1p[:], rdn[:])
    nc.vector.tensor_mul(p2p[:], p2p[:], rdn[:])

    e1 = wtile()
    nc.vector.tensor_scalar(e1[:], p1[:], AXi, None, OP.subtract)
    q1 = wtile()
    nc.vector.scalar_tensor_tensor(q1[:], p1[:], GXi, e1[:], OP.subtract, OP.mult)
    e1s = wtile()
    nc.scalar.square(e1s[:], e1[:])
    e2 = e1  # e1 dead
    nc.vector.tensor_scalar(e2[:], p2[:], AYi, None, OP.subtract)
    e2s = p1  # p1 dead
    nc.scalar.square(e2s[:], e2[:])
    s12 = e2
    nc.vector.tensor_add(s12[:], e1s[:], e2s[:])
    d1p = wtile()
    nc.scalar.sqrt(d1p[:], s12[:])
    c1m = e1s
    nc.vector.tensor_scalar(c1m[:], q1[:], 0.0, None, OP.is_lt)
    m1 = q1
    nc.gpsimd.tensor_mul(m1[:], c1m[:], mju[:])

    g1 = s12
    nc.vector.tensor_scalar(g1[:], p1p[:], AXi, None, OP.subtract)
    g1s = c1m
    nc.scalar.square(g1s[:], g1[:])
    g2 = g1
    nc.vector.tensor_scalar(g2[:], p2p[:], AYi, None, OP.subtract)
    g2s = p2  # p2 dead
    nc.scalar.square(g2s[:], g2[:])
    s34 = g2
    nc.vector.tensor_add(s34[:], g1s[:], g2s[:])
    d2p = wtile()
    nc.scalar.sqrt(d2p[:], s34[:])

    t1 = g1s
    nc.gpsimd.tensor_sub(t1[:], p1p[:], axj_b[:])
    t2 = g2s
    nc.gpsimd.tensor_sub(t2[:], p1p[:], gxj_b[:])
    q2 = p1p  # p1p dead
    nc.gpsimd.tensor_mul(q2[:], t1[:], t2[:])
    c2m = t1
    nc.vector.tensor_scalar(c2m[:], q2[:], 0.0, None, OP.is_lt)
    m2 = t2
    nc.gpsimd.tensor_mul(m2[:], c2m[:], mjl[:])

    # walrus requires integer mask dtype for CopyPredicated
    mu1 = work.tile([R, N], mybir.dt.uint8, tag="mu1", name="mu1")
    mu2 = work.tile([R, N], mybir.dt.uint8, tag="mu2", name="mu2")
    nc.vector.tensor_copy(mu1[:], m1[:])
    nc.vector.tensor_copy(mu2[:], m2[:])

    dist = nums.tile([R, N], F32, tag="dist", name="dist")
    nc.vector.tensor_scalar(dist[:], mju[:], 0.0, DVi, OP.mult, OP.add)
    nc.vector.copy_predicated(dist[:], mu1[:], d1p[:])
    nc.vector.copy_predicated(dist[:], mu2[:], d2p[:])

    # ---- phase 3: transpose dist -> dT33 [33, NCHUNK*128] ----
    dT33 = nums.tile([33, NCHUNK * 128], F32, tag="dT33", name="dT33")
    dT_sb = nums.tile([128, N], F32, tag="dT_sb", name="dT_sb")
    with tc.tile_pool(name="ps_tr", bufs=2, space="PSUM") as ps_tr:
        for tt in range(8):
            ps = ps_tr.tile([128, 128], F32, tag="ps_tr", name="ps_tr")
            nc.tensor.transpose(ps[:], dist[:, tt * 128:(tt + 1) * 128], ident[:])
            nc.vector.tensor_copy(dT_sb[:, tt * 128:(tt + 1) * 128], ps[:])
    # relayout: dT33[p, (4t+b)*128 + i] = dT_sb[32b+p, 128t + i]
    dT33_v = dT33[0:32, :].rearrange("p (c i) -> p c i", i=128)
    dT_sb_v = dT_sb[:, :].rearrange("p (t i) -> p t i", i=128)
    for b in range(4):
        nc.sync.dma_start(dT33_v[:, b::4, :], dT_sb_v[32 * b:32 * (b + 1), :, :])
    nc.sync.dma_start(dT33[32:33, :], t["dvi_rep"])

    # ---- phase 4: main loop over 64 half-chunk windows ----
    ps_logit = ctx.enter_context(tc.tile_pool(name="ps_logit", bufs=2, space="PSUM"))
    ps_ah = ctx.enter_context(tc.tile_pool(name="ps_ah", bufs=2, space="PSUM"))
    sig_pool = ctx.enter_context(tc.tile_pool(name="sig", bufs=3))
    out_pool = ctx.enter_context(tc.tile_pool(name="outsb", bufs=2))

    out_sb = None
    for hh in range(NHALF):
        c, half = hh // 2, hh % 2
        r_, q_ = hh % 32, hh // 32
        oh = onehot[:, r_ * 128:(r_ + 1) * 128]
        lg = ps_logit.tile([R, WFREE], F32, tag="lg", name="lg")
        ah = ps_ah.tile([R, WFREE], F32, tag="ah", name="ah")
        for w in range(2):
            dst = slice(w * 512, (w + 1) * 512)
            src = slice(half * WFREE + w * 512, half * WFREE + (w + 1) * 512)
            nc.tensor.matmul(lg[:, dst], dT33[0:33, c * 128:(c + 1) * 128],
                             rhs33[:, src], start=True, stop=False)
            bsrc = slice(q_ * WFREE + half * 0 + w * 512,
                         q_ * WFREE + w * 512 + 512)
            nc.tensor.matmul(lg[:, dst], oh, B_all[:, bsrc],
                             start=False, stop=True)
            nc.tensor.matmul(ah[:, dst], oh, AH_all[:, bsrc],
                             start=True, stop=True)

        sig = sig_pool.tile([R, WFREE], F32, tag="sig", name="sig")
        nc.scalar.activation(sig[:], lg[:], AF.Sigmoid)

        if hh % OUT_BATCH == 0:
            out_sb = out_pool.tile([R, OUT_BATCH * WFREE], F32, tag="out_sb", name="out_sb")
        seg = slice((hh % OUT_BATCH) * WFREE, (hh % OUT_BATCH + 1) * WFREE)
        nc.vector.tensor_mul(out_sb[:, seg], sig[:], ah[:])
        if hh % OUT_BATCH == OUT_BATCH - 1:
            base = (hh - (OUT_BATCH - 1)) * WFREE
            nc.sync.dma_start(t["out"][:, base:base + OUT_BATCH * WFREE],
                              out_sb[:])


def build_nc():
    nc = bacc.Bacc("TRN2", target_bir_lowering=False, debug=False,
                   enable_asserts=False, num_devices=NCORES)
    t = _declare_tensors(nc)
    with tile.TileContext(nc) as tc:
        with ExitStack() as ctx:
            _build_program(ctx, tc, t)
    nc.compile()
    return nc


def kernel(**inputs):
    prep = _host_prep(**inputs)
    nc = build_nc()
    in_maps = [_core_inputs(prep, c) for c in range(NCORES)]
    res = bass_utils.run_bass_kernel_spmd(nc, in_maps, core_ids=list(range(NCORES)))
    out = np.concatenate([r["out"] for r in res.results], 0).reshape(N, N, H)
    # patch the diagonal (host-computed, uses GH and the diag logit)
    out[np.arange(N), np.arange(N)] = prep["out_diag"]
    return out


if __name__ == "__main__":
    import reference
    inputs = {k: np.asarray(v) for k, v in reference.setup_inputs().items()}
    out = kernel(**inputs)
    print("kernel out", out.shape, out.dtype)



# revision 7
# speedup vs baseline: 5.3247x; 5.3247x over previous
"""Trainium2 Bass kernel for nn_InteractionGate (gnn_message_passing).

Contract: kernel(**inputs) takes the FULL unsharded inputs (as in
reference.setup_inputs()) and returns the FULL [1024, 1024, 64] f32 output.
Internally shards the pairwise row dimension i across 8 NeuronCores
(128 rows each), runs one SPMD Bass/Tile program on cores 0-7, gathers.

Math: with
  W1 = w_gate[0:64], W2 = w_gate[64:128], W3 = w_gate[128:144], W4 = w_gate[144:160]
  u3 = w_dist @ W3, u4 = w_dist @ W4
  B  = AH @ (W1+W2) + b_dist @ (W3+W4) + b_gate          [N,H]
the reference reduces (off-diagonal) to
  out[i,j,h] = AH[j,h] * sigmoid(B[j,h] + diagv[i]*u3[h] + dist[i,j]*u4[h])
where dist is the cal_dist "distance_other" matrix. Diagonal patched on host.

Device plan per core (core owns 128 i-rows; j-partition main loop):
  1. PE computes five pairwise numerator matrices (rank<=6) as K=6 fp32
     matmuls (partition=i, free=j).
  2. DVE/ACT/Pool compute dist[i,j] [128,1024] elementwise; dist is cast
     to fp32r for the main loop.
  3. Main loop over 8 j-blocks x 4 PSUM quarters ([128 j, 2048=(32 i,64 h)]):
     PE (fp32r): lg[j,(i,h)] = dist_block^T-contraction @ delta_u4   (dist*u4)
                             + combo65(B^T|ones) @ RG65(delta_h|G)   (B + diagv*u3)
     ACT: sig = sigmoid(lg) -> bf16  (PSUM -> SBUF)
     DVE: out = sig * ah32 (AH pre-replicated 32x, pre-scaled 8192) -> fp16
     DMA: half-block [128 j, 4096] -> HBM (fp16, 16 KiB rows).
  Output DRAM layout per core: [1024 j, 128 i * 64 h] fp16 scaled by 8192;
  the host transposes to [i, j, h] and divides the scale back out.
"""
import os
import sys
from contextlib import ExitStack

import numpy as np

if "/opt/trn_rl_repo" not in sys.path:
    sys.path.insert(0, "/opt/trn_rl_repo")

import concourse.bass as bass
import concourse.bacc as bacc
import concourse.mybir as mybir
import concourse.tile as tile
from concourse import bass_utils

N, H, E = 1024, 64, 16
NCORES = 8
R = N // NCORES            # 128 rows per core
F32 = mybir.dt.float32
F16 = mybir.dt.float16
BF16 = mybir.dt.bfloat16
F32R = mybir.dt.float32r
AF = mybir.ActivationFunctionType
OP = mybir.AluOpType

NJB = 8                    # j blocks of 128
NQ = 4                     # PSUM quarters per block
QF = 2048                  # free elems per quarter = 32 i * 64 h
BLKF = NQ * QF             # 8192 free elems per block = 128 i * 64 h


def _sigmoid(x):
    return 1.0 / (1.0 + np.exp(-x))


def _host_prep(action_hidden_state, goal_hidden_state, goal, action,
               w_dist, b_dist, w_gate, b_gate):
    f32 = np.float32
    AH = np.ascontiguousarray(action_hidden_state, f32)
    GH = np.ascontiguousarray(goal_hidden_state, f32)
    goal = np.asarray(goal, f32)
    action = np.asarray(action, f32)
    w_dist = np.asarray(w_dist, f32)
    b_dist = np.asarray(b_dist, f32)
    w_gate = np.asarray(w_gate, f32)
    b_gate = np.asarray(b_gate, f32)

    ax, ay = action[:, 0].copy(), action[:, 1].copy()
    gx, gy = goal[:, 0].copy(), goal[:, 1].copy()
    gyx = gy - gx
    diagv = np.sqrt((ax - gx) ** 2 + (ay - gy) ** 2).astype(f32)

    W1, W2 = w_gate[0:H], w_gate[H:2 * H]
    W3, W4 = w_gate[2 * H:2 * H + E], w_gate[2 * H + E:2 * H + 2 * E]
    u3 = (w_dist @ W3).astype(f32)
    u4 = (w_dist @ W4).astype(f32)
    B = (AH @ (W1 + W2) + b_dist @ (W3 + W4) + b_gate).astype(f32)

    one = np.ones(N, f32)
    # rank factors: num[i,j] = sum_k f[k][i] * g[k][j]
    f_cav = np.stack([ax, -ax * gx, -ay, ay * gx])
    g_cav = np.stack([ay * gx, ay, ax * gx, ax])
    f_caz = np.stack([ax, -ax * gy, -ay, ay * gy])
    g_caz = np.stack([ay * gy, ay, ax * gy, ax])
    f_wcg1 = np.stack([gx, -ax * gx]); g_wcg1 = np.stack([ax * gyx, gyx])
    f_wcg2 = np.stack([gyx, -ax * gyx]); g_wcg2 = np.stack([ax * gx, gx])
    f_scg1 = np.stack([gx, -ay * gx]); g_scg1 = np.stack([ax * gyx, gyx])
    f_t2 = np.stack([gyx, -ax * gyx]); g_t2 = np.stack([ay * gx, gx])
    f_dnm = np.stack([one, -ay, -gx, ay * gx, np.zeros(N, f32), np.zeros(N, f32)])
    g_dnm = np.stack([ay * gx, gx, ay, one, np.zeros(N, f32), np.zeros(N, f32)])

    fg = dict(
        dnm=(f_dnm, g_dnm),
        num1=(np.concatenate([f_cav, -f_wcg1]), np.concatenate([g_cav, g_wcg1])),
        num1p=(np.concatenate([f_cav, f_wcg2]), np.concatenate([g_cav, g_wcg2])),
        num2=(np.concatenate([f_caz, -f_scg1]), np.concatenate([g_caz, g_scg1])),
        num2p=(np.concatenate([f_caz, f_t2]), np.concatenate([g_caz, g_t2])),
    )

    logit_diag = (B + (GH - AH) @ W2 + diagv[:, None] * (u3 + u4)).astype(f32)
    out_diag = (GH * _sigmoid(logit_diag)).astype(f32)

    f16 = np.float16
    # --- v6 shared constant tiles (all matmul inputs fp32r: a PSUM
    # accumulation group must keep one PE dtype) ---
    # delta_u4[i', i*64+h] = (i'==i) * u4[h]
    delta_u4 = np.zeros((R, R * H), f32)
    for i in range(R):
        delta_u4[i, i * H:(i + 1) * H] = u4
    # RG65 rows 0..63: delta_h[h', i*64+h] = (h'==h); row 64 = G per core
    RG65_top = np.tile(np.eye(H, dtype=f32), (1, R))  # [64, 8192]
    # combo65: rows 0..63 = B.T; row 64 = ones (G carrier)
    combo65 = np.concatenate([B.T, np.ones((1, N), f32)], 0)
    # AH_T[jp, jb*64+h] = AH[jb*128+jp, h]; ah32 = AH_T block tiled 32x along
    # the i dimension, pre-scaled by 8192 so scaled-fp16 outputs stay in the
    # fp16 normal range (host divides back by 8192 exactly).
    AH_T = np.ascontiguousarray(
        AH.reshape(NJB, R, H).transpose(1, 0, 2).reshape(R, NJB * H))
    ah32 = np.ascontiguousarray(np.tile(
        (AH_T * 8192.0).astype(f16).reshape(R, NJB, 1, H),
        (1, 1, 32, 1)).reshape(R, NJB * 32 * H))

    return dict(AH=AH, GH=GH, ax=ax, ay=ay, gx=gx, gy=gy, diagv=diagv,
                u3=u3, u4=u4, B=B, fg=fg, out_diag=out_diag,
                delta_u4=delta_u4, RG65_top=RG65_top, combo65=combo65,
                ah32=ah32)


NUM_NAMES = ["dnm", "num1", "num1p", "num2", "num2p"]


def _core_inputs(prep, core):
    """Build the per-core in_map (numpy arrays for every ExternalInput)."""
    f32 = np.float32
    i0 = core * R
    sl = slice(i0, i0 + R)

    sc = np.zeros((R, 8), f32)
    sc[:, 0] = prep["ax"][sl]
    sc[:, 1] = prep["ay"][sl]
    sc[:, 2] = prep["gx"][sl]
    sc[:, 3] = prep["diagv"][sl]

    jj = np.arange(N)[None, :]
    ii = np.arange(i0, i0 + R)[:, None]
    mju = (jj > ii).astype(f32)
    mjl = (jj < ii).astype(f32)
    meye = (jj == ii).astype(f32)

    axj_b = np.broadcast_to(prep["ax"], (R, N)).copy()
    gxj_b = np.broadcast_to(prep["gx"], (R, N)).copy()

    G = (prep["diagv"][sl][:, None] * prep["u3"][None, :]).reshape(1, R * H)
    RG65 = np.concatenate([prep["RG65_top"], G.astype(f32)], 0)

    m = dict(sc=sc, mju=mju, mjl=mjl, meye=meye, axj_b=axj_b, gxj_b=gxj_b,
             delta_u4=prep["delta_u4"], RG65=RG65, combo65=prep["combo65"],
             ah32=prep["ah32"])
    for nm in NUM_NAMES:
        f, g = prep["fg"][nm]
        m[f"lhsT_{nm}"] = np.ascontiguousarray(f[:, sl].astype(f32))  # [6, 128]
        m[f"rhs_{nm}"] = np.ascontiguousarray(g.astype(f32))          # [6, 1024]
    return m


def _declare_tensors(nc):
    t = {}
    def inp(name, shape, dt=F32):
        t[name] = nc.dram_tensor(name, shape, dt, kind="ExternalInput").ap()
    inp("sc", [R, 8])
    inp("mju", [R, N]); inp("mjl", [R, N]); inp("meye", [R, N])
    inp("axj_b", [R, N]); inp("gxj_b", [R, N])
    inp("delta_u4", [R, R * H], F32R)
    inp("RG65", [65, R * H], F32R)
    inp("combo65", [65, N], F32R)
    inp("ah32", [R, NJB * 32 * H], F16)
    for nm in NUM_NAMES:
        inp(f"lhsT_{nm}", [6, 128])
        inp(f"rhs_{nm}", [6, N])
    # out[j, i*64+h] per core (j = jb*128+jp); host transposes to [i, j, h]
    t["out"] = nc.dram_tensor("out", [N, R * H], F16, kind="ExternalOutput").ap()
    return t


def _build_program(ctx, tc, t):
    nc = tc.nc

    consts = ctx.enter_context(tc.tile_pool(name="consts", bufs=1))
    distp = ctx.enter_context(tc.tile_pool(name="distp", bufs=1))

    def load_pool(pool, name, shape, dt=F32):
        tl = pool.tile(shape, dt, tag=name, name=name)
        nc.sync.dma_start(tl[:], t[name])
        return tl

    # main-loop constants (persist for the whole program); the big ones are
    # loaded on the ACT hwdge queue so they don't delay phase-1/2 input loads
    # issued on the SP queue.
    sc = load_pool(consts, "sc", [R, 8])
    delta_u4 = consts.tile([R, R * H], F32R, tag="delta_u4", name="delta_u4")
    nc.scalar.dma_start(delta_u4[:], t["delta_u4"])
    RG65 = consts.tile([65, R * H], F32R, tag="RG65", name="RG65")
    nc.scalar.dma_start(RG65[:], t["RG65"])
    combo65 = consts.tile([65, N], F32R, tag="combo65", name="combo65")
    nc.scalar.dma_start(combo65[:], t["combo65"])
    ah32 = consts.tile([R, NJB * 32 * H], F16, tag="ah32", name="ah32")
    nc.scalar.dma_start(ah32[:], t["ah32"])

    AXi, AYi, GXi, DVi = (sc[:, k:k + 1] for k in range(4))

    dist = distp.tile([R, N], F32, tag="dist", name="dist")
    dist_r = distp.tile([R, N], F32R, tag="dist_r", name="dist_r")

    # ---- phases 1+2 in a scratch pool scope (freed before main loop) ----
    with tc.tile_pool(name="p12", bufs=1) as p12, \
         tc.tile_pool(name="work", bufs=1) as work:
        mju = load_pool(p12, "mju", [R, N])
        mjl = load_pool(p12, "mjl", [R, N])
        meye = load_pool(p12, "meye", [R, N])
        axj_b = load_pool(p12, "axj_b", [R, N])
        gxj_b = load_pool(p12, "gxj_b", [R, N])
        lhsT_num = {nm: load_pool(p12, f"lhsT_{nm}", [6, 128])
                    for nm in NUM_NAMES}
        rhs_num = {nm: load_pool(p12, f"rhs_{nm}", [6, N]) for nm in NUM_NAMES}

        # phase 1: numerators via PE (rank<=6), eviction to SBUF
        num_sb = {}
        with tc.tile_pool(name="ps_num", bufs=2, space="PSUM") as ps_num:
            for nm in NUM_NAMES:
                ps = ps_num.tile([R, N], F32, tag="ps_num", name="ps_num")
                for w in range(N // 512):
                    nc.tensor.matmul(ps[:, w * 512:(w + 1) * 512],
                                     lhsT_num[nm][:, :],
                                     rhs_num[nm][:, w * 512:(w + 1) * 512],
                                     start=True, stop=True)
                sb = p12.tile([R, N], F32, tag=f"num_{nm}", name=f"num_{nm}")
                nc.vector.tensor_copy(sb[:], ps[:])
                num_sb[nm] = sb

        # phase 2: dist [128, 1024] elementwise
        def wtile():
            return work.tile([R, N], F32, tag="w", name="w", bufs=8)

        rdn = num_sb["dnm"]
        nc.gpsimd.tensor_add(rdn[:], rdn[:], meye[:])
        nc.vector.reciprocal(rdn[:], rdn[:])
        p1, p2, p1p, p2p = (num_sb[k] for k in ("num1", "num2", "num1p", "num2p"))
        nc.vector.tensor_mul(p1[:], p1[:], rdn[:])
        nc.vector.tensor_mul(p2[:], p2[:], rdn[:])
        nc.vector.tensor_mul(p1p[:], p1p[:], rdn[:])
        nc.vector.tensor_mul(p2p[:], p2p[:], rdn[:])

        e1 = wtile()
        nc.vector.tensor_scalar(e1[:], p1[:], AXi, None, OP.subtract)
        q1 = wtile()
        nc.vector.scalar_tensor_tensor(q1[:], p1[:], GXi, e1[:], OP.subtract, OP.mult)
        e1s = wtile()
        nc.scalar.square(e1s[:], e1[:])
        e2 = e1  # e1 dead
        nc.vector.tensor_scalar(e2[:], p2[:], AYi, None, OP.subtract)
        e2s = p1  # p1 dead
        nc.scalar.square(e2s[:], e2[:])
        s12 = e2
        nc.vector.tensor_add(s12[:], e1s[:], e2s[:])
        d1p = wtile()
        nc.scalar.sqrt(d1p[:], s12[:])
        c1m = e1s
        nc.vector.tensor_scalar(c1m[:], q1[:], 0.0, None, OP.is_lt)
        m1 = q1
        nc.gpsimd.tensor_mul(m1[:], c1m[:], mju[:])

        g1 = s12
        nc.vector.tensor_scalar(g1[:], p1p[:], AXi, None, OP.subtract)
        g1s = c1m
        nc.scalar.square(g1s[:], g1[:])
        g2 = g1
        nc.vector.tensor_scalar(g2[:], p2p[:], AYi, None, OP.subtract)
        g2s = p2  # p2 dead
        nc.scalar.square(g2s[:], g2[:])
        s34 = g2
        nc.vector.tensor_add(s34[:], g1s[:], g2s[:])
        d2p = wtile()
        nc.scalar.sqrt(d2p[:], s34[:])

        t1 = g1s
        nc.gpsimd.tensor_sub(t1[:], p1p[:], axj_b[:])
        t2 = g2s
        nc.gpsimd.tensor_sub(t2[:], p1p[:], gxj_b[:])
        q2 = p1p  # p1p dead
        nc.gpsimd.tensor_mul(q2[:], t1[:], t2[:])
        c2m = t1
        nc.vector.tensor_scalar(c2m[:], q2[:], 0.0, None, OP.is_lt)
        m2 = t2
        nc.gpsimd.tensor_mul(m2[:], c2m[:], mjl[:])

        mu1 = work.tile([R, N], mybir.dt.uint8, tag="mu1", name="mu1")
        mu2 = work.tile([R, N], mybir.dt.uint8, tag="mu2", name="mu2")
        nc.vector.tensor_copy(mu1[:], m1[:])
        nc.vector.tensor_copy(mu2[:], m2[:])

        nc.vector.tensor_scalar(dist[:], mju[:], 0.0, DVi, OP.mult, OP.add)
        nc.vector.copy_predicated(dist[:], mu1[:], d1p[:])
        nc.vector.copy_predicated(dist[:], mu2[:], d2p[:])
        nc.vector.tensor_copy(dist_r[:], dist[:])

    # ---- phase 3: main loop over 8 j-blocks, j-partition layout ----
    ps_pool = ctx.enter_context(tc.tile_pool(name="ps_lg", bufs=2, space="PSUM"))
    sig_pool = ctx.enter_context(tc.tile_pool(name="sig", bufs=4))
    out_pool = ctx.enter_context(tc.tile_pool(name="outsb", bufs=3))

    for jb in range(NJB):
        dist_w = dist_r[:, jb * 128:(jb + 1) * 128]
        combo_w = combo65[:, jb * 128:(jb + 1) * 128]
        ah_q = ah32[:, jb * 2048:(jb + 1) * 2048]
        for half in range(2):
            out_sb = out_pool.tile([R, 2 * QF], F16, tag="out_sb", name="out_sb")
            lgs = []
            for q in range(2):
                qi = half * 2 + q
                base = qi * QF
                lg = ps_pool.tile([R, QF], F32, tag="lg", name="lg")
                lgs.append((lg, base))
            # weight burst: 8x mm_a (one stationary dist block), then 8x mm_bc
            for lg, base in lgs:
                for w in range(4):
                    cs = slice(base + w * 512, base + (w + 1) * 512)
                    nc.tensor.matmul(lg[:, w * 512:(w + 1) * 512], dist_w,
                                     delta_u4[:, cs], start=True, stop=False)
            for lg, base in lgs:
                for w in range(4):
                    cs = slice(base + w * 512, base + (w + 1) * 512)
                    nc.tensor.matmul(lg[:, w * 512:(w + 1) * 512], combo_w,
                                     RG65[:, cs], start=False, stop=True)
            for q, (lg, base) in enumerate(lgs):
                sig = sig_pool.tile([R, QF], BF16, tag="sig", name="sig")
                nc.scalar.activation(sig[:], lg[:], AF.Sigmoid)
                nc.vector.tensor_mul(out_sb[:, q * QF:(q + 1) * QF],
                                     sig[:, :], ah_q)
            nc.sync.dma_start(
                t["out"][jb * 128:(jb + 1) * 128,
                         half * 2 * QF:(half + 1) * 2 * QF],
                out_sb[:])

def build_nc():
    nc = bacc.Bacc("TRN2", target_bir_lowering=False, debug=False,
                   enable_asserts=False, num_devices=NCORES)
    t = _declare_tensors(nc)
    with tile.TileContext(nc) as tc:
        with ExitStack() as ctx:
            _build_program(ctx, tc, t)
    nc.compile()
    return nc


def kernel(**inputs):
    prep = _host_prep(**inputs)
    nc = build_nc()
    in_maps = [_core_inputs(prep, c) for c in range(NCORES)]
    res = bass_utils.run_bass_kernel_spmd(nc, in_maps, core_ids=list(range(NCORES)))
    out = np.empty((N, N, H), np.float32)
    for c in range(NCORES):
        # per-core out: [j, i_local*H + h] (fp16) -> [i_local, j, h] (f32)
        arr = np.asarray(res.results[c]["out"]).reshape(N, R, H)
        out[c * R:(c + 1) * R] = (
            arr.transpose(1, 0, 2).astype(np.float32) * (1.0 / 8192.0))
    # patch the diagonal (host-computed, uses GH and the diag logit)
    out[np.arange(N), np.arange(N)] = prep["out_diag"]
    return out


if __name__ == "__main__":
    import reference
    inputs = {k: np.asarray(v) for k, v in reference.setup_inputs().items()}
    out = kernel(**inputs)
    print("kernel out", out.shape, out.dtype)


# revision 8
# speedup vs baseline: 5.5393x; 1.0403x over previous
"""Trainium2 Bass kernel for nn_InteractionGate (gnn_message_passing).

Contract: kernel(**inputs) takes the FULL unsharded inputs (as in
reference.setup_inputs()) and returns the FULL [1024, 1024, 64] f32 output.
Internally shards the pairwise row dimension i across 8 NeuronCores
(128 rows each), runs one SPMD Bass/Tile program on cores 0-7, gathers.

Math: with
  W1 = w_gate[0:64], W2 = w_gate[64:128], W3 = w_gate[128:144], W4 = w_gate[144:160]
  u3 = w_dist @ W3, u4 = w_dist @ W4
  B  = AH @ (W1+W2) + b_dist @ (W3+W4) + b_gate          [N,H]
the reference reduces (off-diagonal) to
  out[i,j,h] = AH[j,h] * sigmoid(B[j,h] + diagv[i]*u3[h] + dist[i,j]*u4[h])
where dist is the cal_dist "distance_other" matrix. Diagonal patched on host.

Device plan per core (core owns 128 i-rows; j-partition main loop):
  1. PE computes five pairwise numerator matrices (rank<=6) as K=6 fp32
     matmuls (partition=i, free=j); their small inputs are loaded first on
     the ACT hwdge queue so the PE unblocks early.
  2. DVE/ACT/Pool compute dist[i,j] [128,1024] elementwise (approx-accurate
     reciprocal, branch masks via predicated copies); cast to fp32r.
  3. Main loop over 8 j-blocks x 4 PSUM quarters ([128 j, 2048=(32 i,64 h)]):
     PE (fp32r): lg[j,(i,h)] = dist_block^T-contraction @ delta_u4   (dist*u4)
                             + combo65(B^T|ones) @ RG65(delta_h|G)   (B + diagv*u3)
     ACT: sig = sigmoid(lg) -> bf16  (PSUM -> SBUF)
     DVE: out = sig * ah32 (AH pre-replicated 32x, pre-scaled 8192) -> fp16
     DMA: half-block [128 j, 4096] -> HBM (fp16, 16 KiB rows), SP/ACT
     queues alternating.
  Output DRAM layout per core: [1024 j, 128 i * 64 h] fp16 scaled by 8192;
  the host transposes to [i, j, h] and divides the scale back out.
"""
import os
import sys
from contextlib import ExitStack

import numpy as np

if "/opt/trn_rl_repo" not in sys.path:
    sys.path.insert(0, "/opt/trn_rl_repo")

import concourse.bass as bass
import concourse.bacc as bacc
import concourse.mybir as mybir
import concourse.tile as tile
from concourse import bass_utils

N, H, E = 1024, 64, 16
NCORES = 8
R = N // NCORES            # 128 rows per core
F32 = mybir.dt.float32
F16 = mybir.dt.float16
BF16 = mybir.dt.bfloat16
F32R = mybir.dt.float32r
AF = mybir.ActivationFunctionType
OP = mybir.AluOpType

NJB = 8                    # j blocks of 128
NQ = 4                     # PSUM quarters per block
QF = 2048                  # free elems per quarter = 32 i * 64 h
BLKF = NQ * QF             # 8192 free elems per block = 128 i * 64 h


def _sigmoid(x):
    return 1.0 / (1.0 + np.exp(-x))


def _host_prep(action_hidden_state, goal_hidden_state, goal, action,
               w_dist, b_dist, w_gate, b_gate):
    f32 = np.float32
    AH = np.ascontiguousarray(action_hidden_state, f32)
    GH = np.ascontiguousarray(goal_hidden_state, f32)
    goal = np.asarray(goal, f32)
    action = np.asarray(action, f32)
    w_dist = np.asarray(w_dist, f32)
    b_dist = np.asarray(b_dist, f32)
    w_gate = np.asarray(w_gate, f32)
    b_gate = np.asarray(b_gate, f32)

    ax, ay = action[:, 0].copy(), action[:, 1].copy()
    gx, gy = goal[:, 0].copy(), goal[:, 1].copy()
    gyx = gy - gx
    diagv = np.sqrt((ax - gx) ** 2 + (ay - gy) ** 2).astype(f32)

    W1, W2 = w_gate[0:H], w_gate[H:2 * H]
    W3, W4 = w_gate[2 * H:2 * H + E], w_gate[2 * H + E:2 * H + 2 * E]
    u3 = (w_dist @ W3).astype(f32)
    u4 = (w_dist @ W4).astype(f32)
    B = (AH @ (W1 + W2) + b_dist @ (W3 + W4) + b_gate).astype(f32)

    one = np.ones(N, f32)
    # rank factors: num[i,j] = sum_k f[k][i] * g[k][j]
    f_cav = np.stack([ax, -ax * gx, -ay, ay * gx])
    g_cav = np.stack([ay * gx, ay, ax * gx, ax])
    f_caz = np.stack([ax, -ax * gy, -ay, ay * gy])
    g_caz = np.stack([ay * gy, ay, ax * gy, ax])
    f_wcg1 = np.stack([gx, -ax * gx]); g_wcg1 = np.stack([ax * gyx, gyx])
    f_wcg2 = np.stack([gyx, -ax * gyx]); g_wcg2 = np.stack([ax * gx, gx])
    f_scg1 = np.stack([gx, -ay * gx]); g_scg1 = np.stack([ax * gyx, gyx])
    f_t2 = np.stack([gyx, -ax * gyx]); g_t2 = np.stack([ay * gx, gx])
    f_dnm = np.stack([one, -ay, -gx, ay * gx, np.zeros(N, f32), np.zeros(N, f32)])
    g_dnm = np.stack([ay * gx, gx, ay, one, np.zeros(N, f32), np.zeros(N, f32)])

    fg = dict(
        dnm=(f_dnm, g_dnm),
        num1=(np.concatenate([f_cav, -f_wcg1]), np.concatenate([g_cav, g_wcg1])),
        num1p=(np.concatenate([f_cav, f_wcg2]), np.concatenate([g_cav, g_wcg2])),
        num2=(np.concatenate([f_caz, -f_scg1]), np.concatenate([g_caz, g_scg1])),
        num2p=(np.concatenate([f_caz, f_t2]), np.concatenate([g_caz, g_t2])),
    )

    logit_diag = (B + (GH - AH) @ W2 + diagv[:, None] * (u3 + u4)).astype(f32)
    out_diag = (GH * _sigmoid(logit_diag)).astype(f32)

    f16 = np.float16
    # --- v6 shared constant tiles (all matmul inputs fp32r: a PSUM
    # accumulation group must keep one PE dtype) ---
    # delta_u4[i', i*64+h] = (i'==i) * u4[h]
    delta_u4 = np.zeros((R, R * H), f32)
    for i in range(R):
        delta_u4[i, i * H:(i + 1) * H] = u4
    # RG65 rows 0..63: delta_h[h', i*64+h] = (h'==h); row 64 = G per core
    RG65_top = np.tile(np.eye(H, dtype=f32), (1, R))  # [64, 8192]
    # combo65: rows 0..63 = B.T; row 64 = ones (G carrier)
    combo65 = np.concatenate([B.T, np.ones((1, N), f32)], 0)
    # AH_T[jp, jb*64+h] = AH[jb*128+jp, h]; ah32 = AH_T block tiled 32x along
    # the i dimension, pre-scaled by 8192 so scaled-fp16 outputs stay in the
    # fp16 normal range (host divides back by 8192 exactly).
    AH_T = np.ascontiguousarray(
        AH.reshape(NJB, R, H).transpose(1, 0, 2).reshape(R, NJB * H))
    ah32 = np.ascontiguousarray(np.tile(
        (AH_T * 8192.0).astype(f16).reshape(R, NJB, 1, H),
        (1, 1, 32, 1)).reshape(R, NJB * 32 * H))

    return dict(AH=AH, GH=GH, ax=ax, ay=ay, gx=gx, gy=gy, diagv=diagv,
                u3=u3, u4=u4, B=B, fg=fg, out_diag=out_diag,
                delta_u4=delta_u4, RG65_top=RG65_top, combo65=combo65,
                ah32=ah32)


NUM_NAMES = ["dnm", "num1", "num1p", "num2", "num2p"]


def _core_inputs(prep, core):
    """Build the per-core in_map (numpy arrays for every ExternalInput)."""
    f32 = np.float32
    i0 = core * R
    sl = slice(i0, i0 + R)

    sc = np.zeros((R, 8), f32)
    sc[:, 0] = prep["ax"][sl]
    sc[:, 1] = prep["ay"][sl]
    sc[:, 2] = prep["gx"][sl]
    sc[:, 3] = prep["diagv"][sl]

    jj = np.arange(N)[None, :]
    ii = np.arange(i0, i0 + R)[:, None]
    mju = (jj > ii).astype(f32)
    mjl = (jj < ii).astype(f32)
    meye = (jj == ii).astype(f32)

    axj_b = np.broadcast_to(prep["ax"], (R, N)).copy()
    gxj_b = np.broadcast_to(prep["gx"], (R, N)).copy()

    G = (prep["diagv"][sl][:, None] * prep["u3"][None, :]).reshape(1, R * H)
    RG65 = np.concatenate([prep["RG65_top"], G.astype(f32)], 0)

    m = dict(sc=sc, mju=mju, mjl=mjl, meye=meye, axj_b=axj_b, gxj_b=gxj_b,
             delta_u4=prep["delta_u4"], RG65=RG65, combo65=prep["combo65"],
             ah32=prep["ah32"])
    for nm in NUM_NAMES:
        f, g = prep["fg"][nm]
        m[f"lhsT_{nm}"] = np.ascontiguousarray(f[:, sl].astype(f32))  # [6, 128]
        m[f"rhs_{nm}"] = np.ascontiguousarray(g.astype(f32))          # [6, 1024]
    return m


def _declare_tensors(nc):
    t = {}
    def inp(name, shape, dt=F32):
        t[name] = nc.dram_tensor(name, shape, dt, kind="ExternalInput").ap()
    inp("sc", [R, 8])
    inp("mju", [R, N]); inp("mjl", [R, N]); inp("meye", [R, N])
    inp("axj_b", [R, N]); inp("gxj_b", [R, N])
    inp("delta_u4", [R, R * H], F32R)
    inp("RG65", [65, R * H], F32R)
    inp("combo65", [65, N], F32R)
    inp("ah32", [R, NJB * 32 * H], F16)
    for nm in NUM_NAMES:
        inp(f"lhsT_{nm}", [6, 128])
        inp(f"rhs_{nm}", [6, N])
    # out[j, i*64+h] per core (j = jb*128+jp); host transposes to [i, j, h]
    t["out"] = nc.dram_tensor("out", [N, R * H], F16, kind="ExternalOutput").ap()
    return t


def _build_program(ctx, tc, t):
    nc = tc.nc

    consts = ctx.enter_context(tc.tile_pool(name="consts", bufs=1))
    distp = ctx.enter_context(tc.tile_pool(name="distp", bufs=1))

    def load_pool(pool, name, shape, dt=F32):
        tl = pool.tile(shape, dt, tag=name, name=name)
        nc.sync.dma_start(tl[:], t[name])
        return tl

    # Phase-1 numerator tiles load FIRST on the ACT hwdge queue so the
    # first matmuls unblock as early as possible; the big main-loop
    # constants follow on the same queue, and the phase-2 masks ride the
    # SP queue in parallel.
    numsin = ctx.enter_context(tc.tile_pool(name="numsin", bufs=1))
    lhsT_num = {}
    rhs_num = {}
    for nm in NUM_NAMES:
        tl = numsin.tile([6, 128], F32, tag=f"lhsT_{nm}", name=f"lhsT_{nm}")
        nc.scalar.dma_start(tl[:], t[f"lhsT_{nm}"])
        lhsT_num[nm] = tl
        tr = numsin.tile([6, N], F32, tag=f"rhs_{nm}", name=f"rhs_{nm}")
        nc.scalar.dma_start(tr[:], t[f"rhs_{nm}"])
        rhs_num[nm] = tr
    sc = load_pool(consts, "sc", [R, 8])
    delta_u4 = consts.tile([R, R * H], F32R, tag="delta_u4", name="delta_u4")
    nc.scalar.dma_start(delta_u4[:], t["delta_u4"])
    RG65 = consts.tile([65, R * H], F32R, tag="RG65", name="RG65")
    nc.scalar.dma_start(RG65[:], t["RG65"])
    combo65 = consts.tile([65, N], F32R, tag="combo65", name="combo65")
    nc.scalar.dma_start(combo65[:], t["combo65"])
    AXi, AYi, GXi, DVi = (sc[:, k:k + 1] for k in range(4))

    dist = distp.tile([R, N], F32, tag="dist", name="dist")
    dist_r = distp.tile([R, N], F32R, tag="dist_r", name="dist_r")

    ah32p = ctx.enter_context(tc.tile_pool(name="ah32p", bufs=1))

    # ---- phases 1+2 in a scratch pool scope (freed before main loop) ----
    with tc.tile_pool(name="p12", bufs=1) as p12, \
         tc.tile_pool(name="work", bufs=1) as work:
        # phase 1: numerators via PE (rank<=6), eviction to SBUF.
        # Emitted BEFORE the mask loads so the first matmul's DMA-queue
        # semaphore wait only covers the small numerator tiles.
        num_sb = {}
        with tc.tile_pool(name="ps_num", bufs=2, space="PSUM") as ps_num:
            for nm in NUM_NAMES:
                ps = ps_num.tile([R, N], F32, tag="ps_num", name="ps_num")
                for w in range(N // 512):
                    nc.tensor.matmul(ps[:, w * 512:(w + 1) * 512],
                                     lhsT_num[nm][:, :],
                                     rhs_num[nm][:, w * 512:(w + 1) * 512],
                                     start=True, stop=True)
                sb = p12.tile([R, N], F32, tag=f"num_{nm}", name=f"num_{nm}")
                nc.vector.tensor_copy(sb[:], ps[:])
                num_sb[nm] = sb

        meye = load_pool(p12, "meye", [R, N])
        mju = load_pool(p12, "mju", [R, N])
        mjl = load_pool(p12, "mjl", [R, N])
        axj_b = load_pool(p12, "axj_b", [R, N])
        gxj_b = load_pool(p12, "gxj_b", [R, N])
        ah32 = ah32p.tile([R, NJB * 32 * H], F16, tag="ah32", name="ah32")
        nc.sync.dma_start(ah32[:], t["ah32"])

        # phase 2: dist [128, 1024] elementwise
        def wtile():
            return work.tile([R, N], F32, tag="w", name="w", bufs=8)

        rdn = num_sb["dnm"]
        nc.gpsimd.tensor_add(rdn[:], rdn[:], meye[:])
        rscr = wtile()
        nc.vector.reciprocal_approx_accurate(rdn[:], rdn[:], rscr[:])
        p1, p2, p1p, p2p = (num_sb[k] for k in ("num1", "num2", "num1p", "num2p"))
        nc.vector.tensor_mul(p1[:], p1[:], rdn[:])
        nc.vector.tensor_mul(p2[:], p2[:], rdn[:])
        nc.vector.tensor_mul(p1p[:], p1p[:], rdn[:])
        nc.vector.tensor_mul(p2p[:], p2p[:], rdn[:])

        e1 = wtile()
        nc.vector.tensor_scalar(e1[:], p1[:], AXi, None, OP.subtract)
        q1 = wtile()
        nc.vector.scalar_tensor_tensor(q1[:], p1[:], GXi, e1[:], OP.subtract, OP.mult)
        e1s = wtile()
        nc.scalar.square(e1s[:], e1[:])
        e2 = e1  # e1 dead
        nc.vector.tensor_scalar(e2[:], p2[:], AYi, None, OP.subtract)
        e2s = p1  # p1 dead
        nc.scalar.square(e2s[:], e2[:])
        s12 = e2
        nc.vector.tensor_add(s12[:], e1s[:], e2s[:])
        d1p = wtile()
        nc.scalar.sqrt(d1p[:], s12[:])
        c1m = e1s
        nc.vector.tensor_scalar(c1m[:], q1[:], 0.0, None, OP.is_lt)
        m1 = q1
        nc.gpsimd.tensor_mul(m1[:], c1m[:], mju[:])

        g1 = s12
        nc.vector.tensor_scalar(g1[:], p1p[:], AXi, None, OP.subtract)
        g1s = c1m
        nc.scalar.square(g1s[:], g1[:])
        g2 = g1
        nc.vector.tensor_scalar(g2[:], p2p[:], AYi, None, OP.subtract)
        g2s = p2  # p2 dead
        nc.scalar.square(g2s[:], g2[:])
        s34 = g2
        nc.vector.tensor_add(s34[:], g1s[:], g2s[:])
        d2p = wtile()
        nc.scalar.sqrt(d2p[:], s34[:])

        t1 = g1s
        nc.gpsimd.tensor_sub(t1[:], p1p[:], axj_b[:])
        t2 = g2s
        nc.gpsimd.tensor_sub(t2[:], p1p[:], gxj_b[:])
        q2 = p1p  # p1p dead
        nc.gpsimd.tensor_mul(q2[:], t1[:], t2[:])
        c2m = t1
        nc.vector.tensor_scalar(c2m[:], q2[:], 0.0, None, OP.is_lt)
        m2 = t2
        nc.gpsimd.tensor_mul(m2[:], c2m[:], mjl[:])

        mu1 = work.tile([R, N], mybir.dt.uint8, tag="mu1", name="mu1")
        mu2 = work.tile([R, N], mybir.dt.uint8, tag="mu2", name="mu2")
        nc.vector.tensor_copy(mu1[:], m1[:])
        nc.vector.tensor_copy(mu2[:], m2[:])

        nc.vector.tensor_scalar(dist[:], mju[:], 0.0, DVi, OP.mult, OP.add)
        nc.vector.copy_predicated(dist[:], mu1[:], d1p[:])
        nc.vector.copy_predicated(dist[:], mu2[:], d2p[:])
        nc.vector.tensor_copy(dist_r[:], dist[:])

    # ---- phase 3: main loop over 8 j-blocks, j-partition layout ----
    ps_pool = ctx.enter_context(tc.tile_pool(name="ps_lg", bufs=2, space="PSUM"))
    sig_pool = ctx.enter_context(tc.tile_pool(name="sig", bufs=4))
    out_pool = ctx.enter_context(tc.tile_pool(name="outsb", bufs=3))

    for jb in range(NJB):
        dist_w = dist_r[:, jb * 128:(jb + 1) * 128]
        combo_w = combo65[:, jb * 128:(jb + 1) * 128]
        ah_q = ah32[:, jb * 2048:(jb + 1) * 2048]
        for half in range(2):
            out_sb = out_pool.tile([R, 2 * QF], F16, tag="out_sb", name="out_sb")
            lgs = []
            for q in range(2):
                qi = half * 2 + q
                base = qi * QF
                lg = ps_pool.tile([R, QF], F32, tag="lg", name="lg")
                lgs.append((lg, base))
            # weight burst: 8x mm_a (one stationary dist block), then 8x mm_bc
            for lg, base in lgs:
                for w in range(4):
                    cs = slice(base + w * 512, base + (w + 1) * 512)
                    nc.tensor.matmul(lg[:, w * 512:(w + 1) * 512], dist_w,
                                     delta_u4[:, cs], start=True, stop=False)
            for lg, base in lgs:
                for w in range(4):
                    cs = slice(base + w * 512, base + (w + 1) * 512)
                    nc.tensor.matmul(lg[:, w * 512:(w + 1) * 512], combo_w,
                                     RG65[:, cs], start=False, stop=True)
            for q, (lg, base) in enumerate(lgs):
                sig = sig_pool.tile([R, QF], BF16, tag="sig", name="sig")
                nc.scalar.activation(sig[:], lg[:], AF.Sigmoid)
                nc.vector.tensor_mul(out_sb[:, q * QF:(q + 1) * QF],
                                     sig[:, :], ah_q)
            eng = nc.sync if (jb * 2 + half) % 2 == 0 else nc.scalar
            eng.dma_start(
                t["out"][jb * 128:(jb + 1) * 128,
                         half * 2 * QF:(half + 1) * 2 * QF],
                out_sb[:])

def build_nc():
    nc = bacc.Bacc("TRN2", target_bir_lowering=False, debug=False,
                   enable_asserts=False, num_devices=NCORES)
    t = _declare_tensors(nc)
    with tile.TileContext(nc) as tc:
        with ExitStack() as ctx:
            _build_program(ctx, tc, t)
    nc.compile()
    return nc


def kernel(**inputs):
    prep = _host_prep(**inputs)
    nc = build_nc()
    in_maps = [_core_inputs(prep, c) for c in range(NCORES)]
    res = bass_utils.run_bass_kernel_spmd(nc, in_maps, core_ids=list(range(NCORES)))
    out = np.empty((N, N, H), np.float32)
    for c in range(NCORES):
        # per-core out: [j, i_local*H + h] (fp16) -> [i_local, j, h] (f32)
        arr = np.asarray(res.results[c]["out"]).reshape(N, R, H)
        out[c * R:(c + 1) * R] = (
            arr.transpose(1, 0, 2).astype(np.float32) * (1.0 / 8192.0))
    # patch the diagonal (host-computed, uses GH and the diag logit)
    out[np.arange(N), np.arange(N)] = prep["out_diag"]
    return out


if __name__ == "__main__":
    import reference
    inputs = {k: np.asarray(v) for k, v in reference.setup_inputs().items()}
    out = kernel(**inputs)
    print("kernel out", out.shape, out.dtype)


# revision 9
# speedup vs baseline: 5.6768x; 1.0248x over previous
"""Trainium2 Bass kernel for nn_InteractionGate (gnn_message_passing).

Contract: kernel(**inputs) takes the FULL unsharded inputs (as in
reference.setup_inputs()) and returns the FULL [1024, 1024, 64] f32 output.
Internally shards the pairwise row dimension i across 8 NeuronCores
(128 rows each), runs one SPMD Bass/Tile program on cores 0-7, gathers.

Math: with
  W1 = w_gate[0:64], W2 = w_gate[64:128], W3 = w_gate[128:144], W4 = w_gate[144:160]
  u3 = w_dist @ W3, u4 = w_dist @ W4
  B  = AH @ (W1+W2) + b_dist @ (W3+W4) + b_gate          [N,H]
the reference reduces (off-diagonal) to
  out[i,j,h] = AH[j,h] * sigmoid(B[j,h] + diagv[i]*u3[h] + dist[i,j]*u4[h])
where dist is the cal_dist "distance_other" matrix. Diagonal patched on host.

Device plan per core (core owns 128 i-rows; j-partition main loop):
  1. PE computes five pairwise numerator matrices (rank<=6) as K=6 fp32
     matmuls (partition=i, free=j); their small inputs are loaded first on
     the ACT hwdge queue so the PE unblocks early.
  2. DVE/ACT/Pool compute dist[i,j] [128,1024] elementwise (approx-accurate
     reciprocal, branch masks via predicated copies); cast to fp32r.
  3. Main loop over 8 j-blocks x 4 PSUM quarters ([128 j, 2048=(32 i,64 h)]):
     PE (fp32r): lg[j,(i,h)] = dist_block^T-contraction @ delta_u4   (dist*u4)
                             + combo65(B^T|ones) @ RG65(delta_h|G)   (B + diagv*u3)
     (mm_bc is issued first: it has no dist dependency, so the PE
     pre-fills PSUM banks with B+G while the dist chain finishes)
     ACT: sig = sigmoid(lg) -> bf16  (PSUM -> SBUF)
     DVE: out = sig * ah32 (AH pre-replicated 32x, pre-scaled 8192) -> fp16
     DMA: quarter [128 j, 2048] -> HBM (fp16, 8 KiB rows), SP/ACT queues
     alternating.
  Output DRAM layout per core: [1024 j, 128 i * 64 h] fp16 scaled by 8192;
  the host transposes to [i, j, h] and divides the scale back out.
"""
import os
import sys
from contextlib import ExitStack

import numpy as np

if "/opt/trn_rl_repo" not in sys.path:
    sys.path.insert(0, "/opt/trn_rl_repo")

import concourse.bass as bass
import concourse.bacc as bacc
import concourse.mybir as mybir
import concourse.tile as tile
from concourse import bass_utils

N, H, E = 1024, 64, 16
NCORES = 8
R = N // NCORES            # 128 rows per core
F32 = mybir.dt.float32
F16 = mybir.dt.float16
BF16 = mybir.dt.bfloat16
F32R = mybir.dt.float32r
AF = mybir.ActivationFunctionType
OP = mybir.AluOpType

NJB = 8                    # j blocks of 128
NQ = 4                     # PSUM quarters per block
QF = 2048                  # free elems per quarter = 32 i * 64 h
BLKF = NQ * QF             # 8192 free elems per block = 128 i * 64 h


def _sigmoid(x):
    return 1.0 / (1.0 + np.exp(-x))


def _host_prep(action_hidden_state, goal_hidden_state, goal, action,
               w_dist, b_dist, w_gate, b_gate):
    f32 = np.float32
    AH = np.ascontiguousarray(action_hidden_state, f32)
    GH = np.ascontiguousarray(goal_hidden_state, f32)
    goal = np.asarray(goal, f32)
    action = np.asarray(action, f32)
    w_dist = np.asarray(w_dist, f32)
    b_dist = np.asarray(b_dist, f32)
    w_gate = np.asarray(w_gate, f32)
    b_gate = np.asarray(b_gate, f32)

    ax, ay = action[:, 0].copy(), action[:, 1].copy()
    gx, gy = goal[:, 0].copy(), goal[:, 1].copy()
    gyx = gy - gx
    diagv = np.sqrt((ax - gx) ** 2 + (ay - gy) ** 2).astype(f32)

    W1, W2 = w_gate[0:H], w_gate[H:2 * H]
    W3, W4 = w_gate[2 * H:2 * H + E], w_gate[2 * H + E:2 * H + 2 * E]
    u3 = (w_dist @ W3).astype(f32)
    u4 = (w_dist @ W4).astype(f32)
    B = (AH @ (W1 + W2) + b_dist @ (W3 + W4) + b_gate).astype(f32)

    one = np.ones(N, f32)
    # rank factors: num[i,j] = sum_k f[k][i] * g[k][j]
    f_cav = np.stack([ax, -ax * gx, -ay, ay * gx])
    g_cav = np.stack([ay * gx, ay, ax * gx, ax])
    f_caz = np.stack([ax, -ax * gy, -ay, ay * gy])
    g_caz = np.stack([ay * gy, ay, ax * gy, ax])
    f_wcg1 = np.stack([gx, -ax * gx]); g_wcg1 = np.stack([ax * gyx, gyx])
    f_wcg2 = np.stack([gyx, -ax * gyx]); g_wcg2 = np.stack([ax * gx, gx])
    f_scg1 = np.stack([gx, -ay * gx]); g_scg1 = np.stack([ax * gyx, gyx])
    f_t2 = np.stack([gyx, -ax * gyx]); g_t2 = np.stack([ay * gx, gx])
    f_dnm = np.stack([one, -ay, -gx, ay * gx, np.zeros(N, f32), np.zeros(N, f32)])
    g_dnm = np.stack([ay * gx, gx, ay, one, np.zeros(N, f32), np.zeros(N, f32)])

    fg = dict(
        dnm=(f_dnm, g_dnm),
        num1=(np.concatenate([f_cav, -f_wcg1]), np.concatenate([g_cav, g_wcg1])),
        num1p=(np.concatenate([f_cav, f_wcg2]), np.concatenate([g_cav, g_wcg2])),
        num2=(np.concatenate([f_caz, -f_scg1]), np.concatenate([g_caz, g_scg1])),
        num2p=(np.concatenate([f_caz, f_t2]), np.concatenate([g_caz, g_t2])),
    )

    logit_diag = (B + (GH - AH) @ W2 + diagv[:, None] * (u3 + u4)).astype(f32)
    out_diag = (GH * _sigmoid(logit_diag)).astype(f32)

    f16 = np.float16
    # --- v6 shared constant tiles (all matmul inputs fp32r: a PSUM
    # accumulation group must keep one PE dtype) ---
    # delta_u4[i', i*64+h] = (i'==i) * u4[h]
    delta_u4 = np.zeros((R, R * H), f32)
    for i in range(R):
        delta_u4[i, i * H:(i + 1) * H] = u4
    # RG65 rows 0..63: delta_h[h', i*64+h] = (h'==h); row 64 = G per core
    RG65_top = np.tile(np.eye(H, dtype=f32), (1, R))  # [64, 8192]
    # combo65: rows 0..63 = B.T; row 64 = ones (G carrier)
    combo65 = np.concatenate([B.T, np.ones((1, N), f32)], 0)
    # AH_T[jp, jb*64+h] = AH[jb*128+jp, h]; ah32 = AH_T block tiled 32x along
    # the i dimension, pre-scaled by 8192 so scaled-fp16 outputs stay in the
    # fp16 normal range (host divides back by 8192 exactly).
    AH_T = np.ascontiguousarray(
        AH.reshape(NJB, R, H).transpose(1, 0, 2).reshape(R, NJB * H))
    ah32 = np.ascontiguousarray(np.tile(
        (AH_T * 8192.0).astype(f16).reshape(R, NJB, 1, H),
        (1, 1, 32, 1)).reshape(R, NJB * 32 * H))

    return dict(AH=AH, GH=GH, ax=ax, ay=ay, gx=gx, gy=gy, diagv=diagv,
                u3=u3, u4=u4, B=B, fg=fg, out_diag=out_diag,
                delta_u4=delta_u4, RG65_top=RG65_top, combo65=combo65,
                ah32=ah32)


NUM_NAMES = ["dnm", "num1", "num1p", "num2", "num2p"]


def _core_inputs(prep, core):
    """Build the per-core in_map (numpy arrays for every ExternalInput)."""
    f32 = np.float32
    i0 = core * R
    sl = slice(i0, i0 + R)

    sc = np.zeros((R, 8), f32)
    sc[:, 0] = prep["ax"][sl]
    sc[:, 1] = prep["ay"][sl]
    sc[:, 2] = prep["gx"][sl]
    sc[:, 3] = prep["diagv"][sl]

    jj = np.arange(N)[None, :]
    ii = np.arange(i0, i0 + R)[:, None]
    mju = (jj > ii).astype(f32)
    mjl = (jj < ii).astype(f32)
    meye = (jj == ii).astype(f32)

    axj_b = np.broadcast_to(prep["ax"], (R, N)).copy()
    gxj_b = np.broadcast_to(prep["gx"], (R, N)).copy()

    G = (prep["diagv"][sl][:, None] * prep["u3"][None, :]).reshape(1, R * H)
    RG65 = np.concatenate([prep["RG65_top"], G.astype(f32)], 0)

    m = dict(sc=sc, mju=mju, mjl=mjl, meye=meye, axj_b=axj_b, gxj_b=gxj_b,
             delta_u4=prep["delta_u4"], RG65=RG65, combo65=prep["combo65"],
             ah32=prep["ah32"])
    for nm in NUM_NAMES:
        f, g = prep["fg"][nm]
        m[f"lhsT_{nm}"] = np.ascontiguousarray(f[:, sl].astype(f32))  # [6, 128]
        m[f"rhs_{nm}"] = np.ascontiguousarray(g.astype(f32))          # [6, 1024]
    return m


def _declare_tensors(nc):
    t = {}
    def inp(name, shape, dt=F32):
        t[name] = nc.dram_tensor(name, shape, dt, kind="ExternalInput").ap()
    inp("sc", [R, 8])
    inp("mju", [R, N]); inp("mjl", [R, N]); inp("meye", [R, N])
    inp("axj_b", [R, N]); inp("gxj_b", [R, N])
    inp("delta_u4", [R, R * H], F32R)
    inp("RG65", [65, R * H], F32R)
    inp("combo65", [65, N], F32R)
    inp("ah32", [R, NJB * 32 * H], F16)
    for nm in NUM_NAMES:
        inp(f"lhsT_{nm}", [6, 128])
        inp(f"rhs_{nm}", [6, N])
    # out[j, i*64+h] per core (j = jb*128+jp); host transposes to [i, j, h]
    t["out"] = nc.dram_tensor("out", [N, R * H], F16, kind="ExternalOutput").ap()
    return t


def _build_program(ctx, tc, t):
    nc = tc.nc

    consts = ctx.enter_context(tc.tile_pool(name="consts", bufs=1))
    distp = ctx.enter_context(tc.tile_pool(name="distp", bufs=1))

    def load_pool(pool, name, shape, dt=F32):
        tl = pool.tile(shape, dt, tag=name, name=name)
        nc.sync.dma_start(tl[:], t[name])
        return tl

    # Phase-1 numerator tiles load FIRST on the ACT hwdge queue so the
    # first matmuls unblock as early as possible; the big main-loop
    # constants follow on the same queue, and the phase-2 masks ride the
    # SP queue in parallel.
    numsin = ctx.enter_context(tc.tile_pool(name="numsin", bufs=1))
    lhsT_num = {}
    rhs_num = {}
    for nm in NUM_NAMES:
        tl = numsin.tile([6, 128], F32, tag=f"lhsT_{nm}", name=f"lhsT_{nm}")
        nc.scalar.dma_start(tl[:], t[f"lhsT_{nm}"])
        lhsT_num[nm] = tl
        tr = numsin.tile([6, N], F32, tag=f"rhs_{nm}", name=f"rhs_{nm}")
        nc.scalar.dma_start(tr[:], t[f"rhs_{nm}"])
        rhs_num[nm] = tr
    sc = load_pool(consts, "sc", [R, 8])
    delta_u4 = consts.tile([R, R * H], F32R, tag="delta_u4", name="delta_u4")
    nc.scalar.dma_start(delta_u4[:], t["delta_u4"])
    RG65 = consts.tile([65, R * H], F32R, tag="RG65", name="RG65")
    nc.scalar.dma_start(RG65[:], t["RG65"])
    combo65 = consts.tile([65, N], F32R, tag="combo65", name="combo65")
    nc.scalar.dma_start(combo65[:], t["combo65"])
    AXi, AYi, GXi, DVi = (sc[:, k:k + 1] for k in range(4))

    dist = distp.tile([R, N], F32, tag="dist", name="dist")
    dist_r = distp.tile([R, N], F32R, tag="dist_r", name="dist_r")

    ah32p = ctx.enter_context(tc.tile_pool(name="ah32p", bufs=1))

    # ---- phases 1+2 in a scratch pool scope (freed before main loop) ----
    with tc.tile_pool(name="p12", bufs=1) as p12, \
         tc.tile_pool(name="work", bufs=1) as work:
        # phase 1: numerators via PE (rank<=6), eviction to SBUF.
        # Emitted BEFORE the mask loads so the first matmul's DMA-queue
        # semaphore wait only covers the small numerator tiles.
        num_sb = {}
        with tc.tile_pool(name="ps_num", bufs=2, space="PSUM") as ps_num:
            for nm in NUM_NAMES:
                ps = ps_num.tile([R, N], F32, tag="ps_num", name="ps_num")
                for w in range(N // 512):
                    nc.tensor.matmul(ps[:, w * 512:(w + 1) * 512],
                                     lhsT_num[nm][:, :],
                                     rhs_num[nm][:, w * 512:(w + 1) * 512],
                                     start=True, stop=True)
                sb = p12.tile([R, N], F32, tag=f"num_{nm}", name=f"num_{nm}")
                nc.vector.tensor_copy(sb[:], ps[:])
                num_sb[nm] = sb

        meye = load_pool(p12, "meye", [R, N])
        mju = load_pool(p12, "mju", [R, N])
        mjl = load_pool(p12, "mjl", [R, N])
        axj_b = load_pool(p12, "axj_b", [R, N])
        gxj_b = load_pool(p12, "gxj_b", [R, N])
        ah32 = ah32p.tile([R, NJB * 32 * H], F16, tag="ah32", name="ah32")
        nc.sync.dma_start(ah32[:], t["ah32"])

        # phase 2: dist [128, 1024] elementwise
        def wtile():
            return work.tile([R, N], F32, tag="w", name="w", bufs=8)

        rdn = num_sb["dnm"]
        nc.gpsimd.tensor_add(rdn[:], rdn[:], meye[:])
        rscr = wtile()
        nc.vector.reciprocal_approx_accurate(rdn[:], rdn[:], rscr[:])
        p1, p2, p1p, p2p = (num_sb[k] for k in ("num1", "num2", "num1p", "num2p"))
        nc.vector.tensor_mul(p1[:], p1[:], rdn[:])
        nc.vector.tensor_mul(p2[:], p2[:], rdn[:])
        nc.vector.tensor_mul(p1p[:], p1p[:], rdn[:])
        nc.vector.tensor_mul(p2p[:], p2p[:], rdn[:])

        e1 = wtile()
        nc.vector.tensor_scalar(e1[:], p1[:], AXi, None, OP.subtract)
        q1 = wtile()
        nc.vector.scalar_tensor_tensor(q1[:], p1[:], GXi, e1[:], OP.subtract, OP.mult)
        e1s = wtile()
        nc.scalar.square(e1s[:], e1[:])
        e2 = e1  # e1 dead
        nc.vector.tensor_scalar(e2[:], p2[:], AYi, None, OP.subtract)
        e2s = p1  # p1 dead
        nc.scalar.square(e2s[:], e2[:])
        s12 = e2
        nc.vector.tensor_add(s12[:], e1s[:], e2s[:])
        d1p = wtile()
        nc.scalar.sqrt(d1p[:], s12[:])
        c1m = e1s
        nc.vector.tensor_scalar(c1m[:], q1[:], 0.0, None, OP.is_lt)
        m1 = q1
        nc.gpsimd.tensor_mul(m1[:], c1m[:], mju[:])

        g1 = s12
        nc.vector.tensor_scalar(g1[:], p1p[:], AXi, None, OP.subtract)
        g1s = c1m
        nc.scalar.square(g1s[:], g1[:])
        g2 = g1
        nc.vector.tensor_scalar(g2[:], p2p[:], AYi, None, OP.subtract)
        g2s = p2  # p2 dead
        nc.scalar.square(g2s[:], g2[:])
        s34 = g2
        nc.vector.tensor_add(s34[:], g1s[:], g2s[:])
        d2p = wtile()
        nc.scalar.sqrt(d2p[:], s34[:])

        t1 = g1s
        nc.gpsimd.tensor_sub(t1[:], p1p[:], axj_b[:])
        t2 = g2s
        nc.gpsimd.tensor_sub(t2[:], p1p[:], gxj_b[:])
        q2 = p1p  # p1p dead
        nc.gpsimd.tensor_mul(q2[:], t1[:], t2[:])
        c2m = t1
        nc.vector.tensor_scalar(c2m[:], q2[:], 0.0, None, OP.is_lt)
        m2 = t2
        nc.gpsimd.tensor_mul(m2[:], c2m[:], mjl[:])

        mu1 = work.tile([R, N], mybir.dt.uint8, tag="mu1", name="mu1")
        mu2 = work.tile([R, N], mybir.dt.uint8, tag="mu2", name="mu2")
        nc.vector.tensor_copy(mu1[:], m1[:])
        nc.vector.tensor_copy(mu2[:], m2[:])

        nc.vector.tensor_scalar(dist[:], mju[:], 0.0, DVi, OP.mult, OP.add)
        nc.vector.copy_predicated(dist[:], mu1[:], d1p[:])
        nc.vector.copy_predicated(dist[:], mu2[:], d2p[:])
        nc.vector.tensor_copy(dist_r[:], dist[:])

    # ---- phase 3: main loop over 8 j-blocks, j-partition layout ----
    ps_pool = ctx.enter_context(tc.tile_pool(name="ps_lg", bufs=2, space="PSUM"))
    sig_pool = ctx.enter_context(tc.tile_pool(name="sig", bufs=4))
    out_pool = ctx.enter_context(tc.tile_pool(name="outsb", bufs=4))

    for jb in range(NJB):
        dist_w = dist_r[:, jb * 128:(jb + 1) * 128]
        combo_w = combo65[:, jb * 128:(jb + 1) * 128]
        ah_q = ah32[:, jb * 2048:(jb + 1) * 2048]
        for half in range(2):
            lgs = []
            for q in range(2):
                qi = half * 2 + q
                base = qi * QF
                lg = ps_pool.tile([R, QF], F32, tag="lg", name="lg")
                lgs.append((lg, base))
            # weight burst, mm_bc FIRST: it has no dist dependency, so the
            # PE can pre-fill PSUM with B+G while the dist chain finishes;
            # mm_a accumulates the dist*u4 term and closes the group.
            for lg, base in lgs:
                for w in range(4):
                    cs = slice(base + w * 512, base + (w + 1) * 512)
                    nc.tensor.matmul(lg[:, w * 512:(w + 1) * 512], combo_w,
                                     RG65[:, cs], start=True, stop=False)
            for lg, base in lgs:
                for w in range(4):
                    cs = slice(base + w * 512, base + (w + 1) * 512)
                    nc.tensor.matmul(lg[:, w * 512:(w + 1) * 512], dist_w,
                                     delta_u4[:, cs], start=False, stop=True)
            for q, (lg, base) in enumerate(lgs):
                sig = sig_pool.tile([R, QF], BF16, tag="sig", name="sig")
                nc.scalar.activation(sig[:], lg[:], AF.Sigmoid)
                out_sb = out_pool.tile([R, QF], F16, tag="out_sb", name="out_sb")
                nc.vector.tensor_mul(out_sb[:], sig[:, :], ah_q)
                qi = half * 2 + q
                eng = nc.sync if (jb * NQ + qi) % 2 == 0 else nc.scalar
                eng.dma_start(
                    t["out"][jb * 128:(jb + 1) * 128,
                             qi * QF:(qi + 1) * QF],
                    out_sb[:])

def build_nc():
    nc = bacc.Bacc("TRN2", target_bir_lowering=False, debug=False,
                   enable_asserts=False, num_devices=NCORES)
    t = _declare_tensors(nc)
    with tile.TileContext(nc) as tc:
        with ExitStack() as ctx:
            _build_program(ctx, tc, t)
    nc.compile()
    return nc


def kernel(**inputs):
    prep = _host_prep(**inputs)
    nc = build_nc()
    in_maps = [_core_inputs(prep, c) for c in range(NCORES)]
    res = bass_utils.run_bass_kernel_spmd(nc, in_maps, core_ids=list(range(NCORES)))
    out = np.empty((N, N, H), np.float32)
    for c in range(NCORES):
        # per-core out: [j, i_local*H + h] (fp16) -> [i_local, j, h] (f32)
        arr = np.asarray(res.results[c]["out"]).reshape(N, R, H)
        out[c * R:(c + 1) * R] = (
            arr.transpose(1, 0, 2).astype(np.float32) * (1.0 / 8192.0))
    # patch the diagonal (host-computed, uses GH and the diag logit)
    out[np.arange(N), np.arange(N)] = prep["out_diag"]
    return out


if __name__ == "__main__":
    import reference
    inputs = {k: np.asarray(v) for k, v in reference.setup_inputs().items()}
    out = kernel(**inputs)
    print("kernel out", out.shape, out.dtype)
